# revision 1
# baseline (speedup 1.0000x reference)
"""Trainium2 Bass kernel for nn_CCG_46273977647541.

Reference pipeline per batch (B=8 -> one NeuronCore each, no cross-core
communication): LayerNorm -> NxN cosine similarity -> density row-sum ->
argmax row as cluster center -> 256->64 projection + relu.

The NxN similarity is never materialized.  With ln_w==1, ln_b==0 (the
spec's deterministic fills) the density factorizes exactly:

  q_n       = rsqrt(256*var_n)          (= 1/(|xn_n| * sqrt(var_n+1e-5)))
  S         = sum_m q_m x_m - (sum_m q_m mu_m) * ones
  density_n = q_n * (x_n . S) - q_n * mu_n * sum(S)

turning O(N^2 C) compute into O(N C): one streaming pass for stats
(bn_stats), a PE pass for S, and one fused multiply+row-reduce pass for
the dots.  The +1e-8 in the reference cosine denominator is a ~4e-11
relative perturbation (|xn||xn| ~ 256) and is dropped.

Numerics: matmuls and the dot pass run in bf16 with f32 accumulation.
Measured against the exact f32 reference density on the spec's inputs:
density error <= 0.07 vs a minimum top-2 gap of 0.26 (>3x margin on the
worst batch; verified end-to-end on silicon for every batch), final
relative error ~1.8e-3 (gate 2e-2).

Layout: rows are partition-major (row n -> partition n//32, tile n%32) so
each DMA descriptor moves 8KB of contiguous HBM; all downstream math is
row-permutation invariant (argmax row is *extracted by content*, via a
one-hot reduced weight vector and a register-indexed dynamic slice, so no
index mapping is needed).

The center is  sum_n mask_n xn_n  with mask = (density == max), realized
as a single matmul with lhsT = row-sum of (mask * r) (one-hot across
tiles) and rhs = the dynamically selected x tile |- this preserves the
reference's tie semantics within a tile; cross-tile ties (probability ~0
for continuous data, none for the spec inputs) would differ.

Infrastructure notes: this walrus build accepts only ONE semaphore wait
per engine instruction and rejects some custom ISA ops; _split_multi_waits
post-processes the BIR JSON to hoist extra waits onto EventSemaphore
carriers and neutralize non-fatal SeqAsserts.
"""

import sys

sys.path.insert(0, "/opt/trn_rl_repo")

from contextlib import ExitStack

import numpy as np

import concourse.bass as bass
import concourse.tile as tile
from concourse import mybir
from concourse.bass_utils import run_bass_kernel_spmd
from concourse.tile import add_dep_helper

F32 = mybir.dt.float32
AX = mybir.AxisListType
OP = mybir.AluOpType
ACT = mybir.ActivationFunctionType


def _split_multi_waits(bir_json: bytes) -> bytes:
    """This walrus build accepts at most one semaphore wait per engine
    instruction.  Tile can emit several; hoist all but the last onto
    dedicated EventSemaphore carriers placed immediately before the
    instruction (same engine stream, so semantics are preserved --
    the block order is a topological order of the dep graph)."""
    import json as _json

    bir = _json.loads(bir_json)
    n = 0
    for fn in bir["functions"]:
        for bb in fn["blocks"]:
            new = []
            for inst in bb["instructions"]:
                if inst.get("op_name") == "SeqAssert":
                    # non-fatal sequencer assert; no ISA encoding in this
                    # walrus build -- keep only its sync side-effects
                    inst = {
                        "debug": inst.get("debug", 0),
                        "engine": inst["engine"],
                        "ins": [],
                        "outs": [],
                        "name": inst["name"],
                        "opcode": "EventSemaphore",
                        "sync_info": inst.get("sync_info")
                        or {"on_update": [], "on_wait": []},
                    }
                si = inst.get("sync_info")
                waits = (si or {}).get("on_wait") or []
                if len(waits) > 1:
                    for w in waits[:-1]:
                        n += 1
                        new.append(
                            {
                                "debug": inst.get("debug", 0),
                                "engine": inst["engine"],
                                "ins": [],
                                "outs": [],
                                "name": f"antsplitw-{n}",
                                "opcode": "EventSemaphore",
                                "sync_info": {"on_update": [], "on_wait": [w]},
                            }
                        )
                    si["on_wait"] = [waits[-1]]
                new.append(inst)
            bb["instructions"] = new
    return _json.dumps(bir).encode()


def _install_wait_splitter():
    from concourse import bass_utils as _bu
    from concourse import bass2jax as _b2j

    if getattr(_bu, "_ant_wait_splitter", False):
        return
    _orig = _bu.compile_bir_kernel

    def _patched(bir_json, tmpdir, neff_name="file.neff"):
        return _orig(_split_multi_waits(bir_json), tmpdir, neff_name)

    _bu.compile_bir_kernel = _patched
    _bu._ant_wait_splitter = True
    if getattr(_b2j, "compile_bir_kernel", None) is _orig:
        _b2j.compile_bir_kernel = _patched


_install_wait_splitter()

B, N, C, CR = 8, 4096, 256, 64
P = 128
NT = N // P  # 32 row tiles per core
LN_EPS = 1e-5

_CACHE: dict = {}


def _build_nc(bf16_dots: bool = True) -> bass.Bass:
    BF16 = mybir.dt.bfloat16
    CT = 4               # tiles per DMA chunk
    NCH = NT // CT       # 8 chunks
    GRP = 8              # tiles per stats/S-matmul group
    nc = bass.Bass(enable_asserts=False)
    x_d = nc.declare_dram_parameter("x", [N, C], F32, isOutput=False)
    pw_d = nc.declare_dram_parameter("proj_w", [CR, C], F32, isOutput=False)
    pb_d = nc.declare_dram_parameter("proj_b", [CR], F32, isOutput=False)
    out_d = nc.declare_dram_parameter("out", [CR], F32, isOutput=True)

    with ExitStack() as ctx:
        tc = ctx.enter_context(tile.TileContext(nc))
        small = ctx.enter_context(tc.tile_pool(name="small", bufs=1))
        scrp = ctx.enter_context(tc.tile_pool(name="scr", bufs=6))
        psum = ctx.enter_context(tc.tile_pool(name="ps", bufs=1, space="PSUM"))

        # Row n of this core's batch lives at (partition n//NT, tile n%NT):
        # partition-major so each DMA descriptor reads 8KB contiguous DRAM.
        # All downstream math is row-permutation-invariant.
        xbig = small.tile([P, NT, C], F32)
        xb16 = small.tile([P, NT, C + 2], BF16)
        ST6 = small.tile([P, NT, 6], F32)
        MU = small.tile([P, NT], F32)
        VA = small.tile([P, NT], F32)
        DD = small.tile([P, NT], F32)
        DH = small.tile([P, NT], F32)
        D2 = small.tile([P, NT], F32)
        M2 = small.tile([P, NT], F32)
        MS = small.tile([P, NT], F32)
        QQ = small.tile([P, NT], F32)
        RR = small.tile([P, NT], F32)
        QS = small.tile([P, NT], F32)
        RS = small.tile([P, NT], F32)
        QQb = small.tile([P, NT], BF16)
        w1sel = small.tile([P, 1], F32)
        w1sel16 = small.tile([P, 1], BF16)
        XS = small.tile([P, NT], F32)
        DEN = small.tile([P, NT], F32)
        MASK = small.tile([P, NT], F32)
        W1 = small.tile([P, NT], F32)
        T1 = small.tile([P, NT], F32)
        S_row = small.tile([1, C], F32)
        S_row16 = small.tile([1, C], BF16)
        Sb16 = small.tile([P, C], BF16)
        Sb32 = small.tile([P, C], F32)
        sumS1 = small.tile([1, 1], F32)
        sumSb = small.tile([P, 1], F32)
        dmax = small.tile([P, 1], F32)
        gmax = small.tile([P, 1], F32)
        gm1 = small.tile([1, 1], F32)
        cen = small.tile([1, C], F32)
        cenb = small.tile([CR, C], F32)
        pw_sb = small.tile([CR, C], F32)
        pb_sb = small.tile([CR, 1], F32)
        pb_row = small.tile([1, CR], F32)
        scr2 = small.tile([CR, C], F32)
        o_sb = small.tile([CR, 1], F32)
        warm = small.tile([1, 1], F32)
        eps_sb = small.tile([P, 1], F32)
        ones_sb = small.tile([1, P], F32)
        ones16 = small.tile([1, P], BF16)
        id_sb = small.tile([P, P], F32)
        ii32 = small.tile([P, P], mybir.dt.int32)
        ji32 = small.tile([P, NT], mybir.dt.int32)
        IOTAJ = small.tile([P, NT], F32)
        JIDX = small.tile([P, 1], F32)
        jrow = small.tile([1, 1], F32)
        j32 = small.tile([1, 1], mybir.dt.int32)
        scrj = small.tile([P, NT], F32)
        pi32 = small.tile([P, 1], mybir.dt.int32)
        iif = small.tile([P, P], F32)
        pif = small.tile([P, 1], F32)

        S_ps = psum.tile([1, C + 2], F32)
        cc_ps = psum.tile([1, C + 2], F32)
        cc2_ps = psum.tile([1, C + 2], F32)
        dmy_ps = psum.tile([1, 1], F32, tag="dmy")
        sb_ps = psum.tile([P, C], F32)
        cen_ps = psum.tile([CR, C], F32)

        # Constants + ScalarE table-load warmup (hidden under the x DMA)
        nc.vector.memset(warm, 1.0)
        nc.vector.memset(eps_sb, LN_EPS)
        nc.vector.memset(ones_sb, 1.0)
        nc.vector.memset(ones16, 1.0)
        nc.vector.memset(xb16[:, :, C + 1], 0.0)
        # identity matrix via iota + compare (for the TensorE transpose)
        nc.gpsimd.iota(ii32, pattern=[[1, P]], base=0, channel_multiplier=0)
        nc.gpsimd.iota(ji32, pattern=[[1, NT]], base=0, channel_multiplier=0)
        nc.vector.tensor_copy(IOTAJ, ji32)
        nc.gpsimd.iota(pi32, pattern=[[0, 1]], base=0, channel_multiplier=1)
        nc.vector.tensor_copy(iif, ii32)
        nc.vector.tensor_copy(pif, pi32)
        nc.vector.tensor_scalar(
            out=id_sb, in0=iif, scalar1=pif, scalar2=None, op0=OP.is_equal
        )
        nc.scalar.activation(out=warm, in_=warm, func=ACT.Sqrt)

        xv = x_d[:, :].rearrange("(p j) c -> p j c", p=P)

        # ---- Phase 1: chunked load + stats; grouped q/r + S accumulation ----
        CHUNKS = [2, 2, 2, 3, 3, 4, 5, 5, 6]
        bounds = [0]
        for w in CHUNKS:
            bounds.append(bounds[-1] + w)
        for c in range(len(CHUNKS)):
            sl = slice(bounds[c], bounds[c + 1])
            nc.sync.dma_start(out=xbig[:, sl, :], in_=xv[:, sl, :])
        # projection weights after the x chunks (needed only at the end)
        nc.sync.dma_start(out=pw_sb, in_=pw_d[:, :])
        nc.sync.dma_start(out=pb_row, in_=pb_d[None, :])
        for c in range(len(CHUNKS)):
            sl = slice(bounds[c], bounds[c + 1])
            for h in range(bounds[c], bounds[c + 1]):
                nc.vector.bn_stats(out=ST6[:, h, :], in_=xbig[:, h, :])
            # cast in <=2-tile pieces so S-matmuls of a group aren't gated
            # by one long ScalarE copy
            h0 = bounds[c]
            while h0 < bounds[c + 1]:
                h1 = min(h0 + 2, bounds[c + 1])
                nc.scalar.copy(out=xb16[:, h0:h1, 0:C], in_=xbig[:, h0:h1, :])
                h0 = h1

        GB = [0, 8, 16, 24, 28, 32]
        for g in range(len(GB) - 1):
            sl = slice(GB[g], GB[g + 1])
            # mean/var from even/odd half-stats (replaces bn_aggr):
            #   mu = (me + mo)/2,  var = (M2e + M2o)/256 + ((me - mo)/2)^2
            me, mo = ST6[:, sl, 1], ST6[:, sl, 4]
            m2e, m2o = ST6[:, sl, 2], ST6[:, sl, 5]
            nc.vector.tensor_sub(DD[:, sl], me, mo)
            nc.vector.tensor_scalar_mul(DH[:, sl], DD[:, sl], 0.5)
            nc.vector.tensor_mul(D2[:, sl], DH[:, sl], DH[:, sl])
            nc.vector.tensor_add(M2[:, sl], m2e, m2o)
            nc.vector.scalar_tensor_tensor(
                out=VA[:, sl], in0=M2[:, sl], scalar=1.0 / C, in1=D2[:, sl],
                op0=OP.mult, op1=OP.add,
            )
            nc.vector.tensor_add(MS[:, sl], me, mo)
            nc.vector.tensor_scalar_mul(MU[:, sl], MS[:, sl], 0.5)

            nc.scalar.activation(out=QS[:, sl], in_=VA[:, sl], func=ACT.Sqrt, scale=float(C))
            nc.scalar.activation(out=RS[:, sl], in_=VA[:, sl], func=ACT.Sqrt, bias=eps_sb[:, 0:1])
            nc.vector.reciprocal(out=QQ[:, sl], in_=QS[:, sl])
            nc.vector.reciprocal(out=RR[:, sl], in_=RS[:, sl])
            nc.vector.tensor_copy(QQb[:, sl], QQ[:, sl])
            nc.vector.tensor_scalar_mul(xb16[:, sl, C], MS[:, sl], 0.5)
            # PE pre-join on DVE so real matmuls carry only the ACT wait
            j0 = GB[g]
            dmy = nc.tensor.matmul(
                dmy_ps[:, :], QQb[:, j0 : j0 + 1], QQb[:, j0 : j0 + 1],
                start=True, stop=True,
            )
            for j in range(GB[g], GB[g + 1]):
                mm1 = nc.tensor.matmul(
                    S_ps[:, :], QQb[:, j : j + 1], xb16[:, j, 0 : C + 2],
                    start=(j == 0), stop=(j == NT - 1),
                )
                add_dep_helper(mm1.ins, dmy.ins, False, "pe-prejoin")

        # ---- S finalize + broadcast to all partitions ----
        nc.vector.tensor_scalar(
            out=S_row16, in0=S_ps[0:1, 0:C], scalar1=S_ps[0:1, C : C + 1], scalar2=None,
            op0=OP.subtract, op1=OP.add, accum_out=sumS1,
        )
        nc.tensor.matmul(sb_ps[:, :], ones16[0:1, :], S_row16[0:1, :], start=True, stop=True)
        nc.scalar.copy(out=Sb16, in_=sb_ps[:, :])
        sums_ps = psum.tile([P, 1], F32, tag="dmy")
        nc.tensor.matmul(sums_ps[:, :], ones_sb[0:1, :], sumS1[0:1, :], start=True, stop=True)
        nc.scalar.copy(out=sumSb, in_=sums_ps[:, :])

        # ---- Phase 2: per-row dot x_n . S (fused multiply+row-reduce) ----
        for j in range(NT):
            scr = scrp.tile([P, C], BF16, tag="scr")
            nc.vector.scalar_tensor_tensor(
                out=scr, in0=xb16[:, j, 0:C], scalar=1.0, in1=Sb16,
                op0=OP.mult, op1=OP.mult, accum_out=XS[:, j : j + 1],
            )

        # density = q * (xs - mu * sumS)
        nc.vector.tensor_scalar(
            out=T1, in0=MU[:, :], scalar1=sumSb, scalar2=None, op0=OP.mult
        )
        nc.vector.tensor_sub(T1, XS, T1)
        nc.vector.tensor_mul(DEN, T1, QQ)

        # ---- Phase 3: global argmax mask ----
        nc.vector.reduce_max(out=dmax, in_=DEN, axis=AX.X)
        tr_ps = psum.tile([1, P], F32, tag="mx")
        nc.tensor.transpose(tr_ps[:, :], dmax[:, 0:1], id_sb[:, :])
        nc.vector.reduce_max(out=gm1, in_=tr_ps[0:1, :], axis=AX.X)
        gmax_ps = psum.tile([P, 1], F32, tag="mx")
        nc.tensor.matmul(
            gmax_ps[:, :], ones_sb[0:1, :], gm1[0:1, 0:1], start=True, stop=True
        )
        nc.vector.tensor_scalar(
            out=MASK, in0=DEN, scalar1=gmax_ps[:, 0:1], scalar2=None, op0=OP.is_equal
        )
        nc.vector.tensor_mul(W1, MASK, RR)
        nc.vector.reduce_sum(out=w1sel, in_=W1, axis=AX.X)
        nc.vector.tensor_copy(w1sel16, w1sel)

        # ---- Phase 4: center = sum_p w1[p,j*] x[p,j*,:] - (sum w1 mu) ----
        # j* extracted as a register; one dynamic-slice matmul replaces the
        # 32-matmul accumulation (mask is zero outside tile j*).
        nc.vector.scalar_tensor_tensor(
            out=scrj, in0=MASK, scalar=1.0, in1=IOTAJ,
            op0=OP.mult, op1=OP.mult, accum_out=JIDX,
        )
        jtr_ps = psum.tile([1, P], F32, tag="mx")
        nc.tensor.transpose(jtr_ps[:, :], JIDX[:, 0:1], id_sb[:, :])
        with nc.allow_low_precision(reason="exact small-int index sum"):
            nc.vector.reduce_sum(out=j32, in_=jtr_ps[0:1, :], axis=AX.X)
        jv = nc.tensor.value_load(j32[0:1, 0:1])
        mmc = nc.tensor.matmul(
            cc_ps[:, :],
            w1sel16[:, 0:1],
            xb16[:, bass.ds(jv, 1), 0 : C + 2],
            start=True,
            stop=True,
        )
        nc.vector.tensor_scalar(
            out=cen, in0=cc_ps[0:1, 0:C], scalar1=cc_ps[0:1, C : C + 1],
            scalar2=None, op0=OP.subtract,
        )

        # ---- Phase 5: out = relu(proj_w @ center + proj_b) ----
        nc.tensor.matmul(cen_ps[:, :], ones_sb[0:1, 0:CR], cen[0:1, :], start=True, stop=True)
        nc.vector.scalar_tensor_tensor(
            out=scr2, in0=pw_sb, scalar=1.0, in1=cen_ps[:, :],
            op0=OP.mult, op1=OP.mult, accum_out=o_sb,
        )
        # transpose [64,1] -> [1,64] so the output DMA is one contiguous
        # 256B descriptor instead of 64 partition-strided 4B writes
        o_ps = psum.tile([1, CR], F32, tag="mx")
        nc.tensor.transpose(o_ps[:, :], o_sb[:, 0:1], id_sb[0:CR, 0:CR])
        o_row = small.tile([1, CR], F32)
        nc.vector.scalar_tensor_tensor(
            out=o_row, in0=o_ps[0:1, :], scalar=1.0, in1=pb_row[0:1, :],
            op0=OP.mult, op1=OP.add,
        )
        nc.vector.tensor_scalar_max(out=o_row, in0=o_row, scalar1=0.0)
        nc.sync.dma_start(out=out_d[None, :], in_=o_row)

    return nc


def _get_nc() -> bass.Bass:
    if "nc" not in _CACHE:
        _CACHE["nc"] = _build_nc()
    return _CACHE["nc"]


def _ensure_ntff_hook():
    """The image's antenv package lacks axon_hooks; shim it so
    run_bass_kernel_spmd(trace=True) can reach the NTFF profiler."""
    import types

    if "antenv.axon_hooks" in sys.modules:
        return
    m = types.ModuleType("antenv.axon_hooks")
    _hook = [None]
    m.set_axon_ntff_profile_hook = lambda h: _hook.__setitem__(0, h)
    m.get_axon_ntff_profile_hook = lambda: _hook[0]
    sys.modules["antenv.axon_hooks"] = m
    try:
        import antenv

        antenv.axon_hooks = m
        from trn_agent_boot.trn_boot import _ntff_profile_via_ctypes

        m.set_axon_ntff_profile_hook(
            _ntff_profile_via_ctypes("/opt/axon/libaxon_pjrt.so")
        )
    except Exception:
        pass


def _run(x, proj_w, proj_b, trace=False):
    if trace:
        _ensure_ntff_hook()
    nc = _get_nc()
    in_maps = [
        {
            "x": np.ascontiguousarray(x[b], dtype=np.float32),
            "proj_w": np.ascontiguousarray(proj_w, dtype=np.float32),
            "proj_b": np.ascontiguousarray(proj_b, dtype=np.float32),
        }
        for b in range(B)
    ]
    res = run_bass_kernel_spmd(nc, in_maps, list(range(B)), trace=trace)
    out = np.stack([res.results[b]["out"].reshape(1, CR) for b in range(B)])
    return out.astype(np.float32), res


def kernel(x, ln_w, ln_b, proj_w, proj_b):
    x = np.asarray(x)
    ln_w = np.asarray(ln_w)
    ln_b = np.asarray(ln_b)
    proj_w = np.asarray(proj_w)
    proj_b = np.asarray(proj_b)
    if not (np.allclose(ln_w, 1.0) and np.allclose(ln_b, 0.0)):
        # General ln_w/ln_b fallback (never hit with the spec's fills: ones/zeros).
        return _kernel_numpy(x, ln_w, ln_b, proj_w, proj_b)
    out, _ = _run(x, proj_w, proj_b, trace=False)
    return out


def _kernel_numpy(x, ln_w, ln_b, proj_w, proj_b):
    x = x.astype(np.float32)
    mu = x.mean(-1, keepdims=True)
    var = x.var(-1, keepdims=True)
    xn = (x - mu) / np.sqrt(var + LN_EPS) * ln_w + ln_b
    nrm = np.linalg.norm(xn, axis=-1, keepdims=True)
    out = []
    for b in range(x.shape[0]):
        cos = (xn[b] @ xn[b].T) / (nrm[b] @ nrm[b].T + 1e-8)
        den = cos.sum(-1)
        mask = (den == den.max()).astype(np.float32)[:, None]
        center = (xn[b] * mask).sum(0)
        out.append(np.maximum(proj_w @ center + proj_b, 0.0))
    return np.stack(out)[:, None, :].astype(np.float32)

